# revision 2
# baseline (speedup 1.0000x reference)
"""BREWA (bit-witness) attention on 8 TRN2 NeuronCores.

Sharding: core c = (batch b, head-group g) with b = c // 2, g = c % 2.
Each core computes its batch's attention for 8 of the 16 heads plus the
partial output projection over those heads' Wo columns; the host sums the
two partial projections per batch (the "all-reduce" is 2-way, done on CPU).

Per-core dataflow (all matmuls bf16, fp32 PSUM accumulate):
  xT[b] (host-transposed, bf16)  --PE-->  QT,KT [512,2048] (dims on partitions)
                                 --PE-->  V    [2048,520]  (seq on partitions,
                                                            65 cols/head: 64 V dims + ones)
  QT,KT --PE (K=64, row+col tiled)--> enc psum --ACT tanh--> q_encT,k_encT
        [128, 2048] tiles: 4 heads x 32 bits on partitions, seq on free dim
  per (head-quad, q-tile 512, k-tile 128):
    ST[k,q] via 4 row-tiled K=32 matmuls -> st psum [128, 2048] (head r at 512r)
    exp(ST/sqrt32) on ACT -> SBUF bf16   (softmax w/o max-sub: |scores|<=5.66)
    att[r] += V_aug[kt,h].T @ expST      (psum [65,512]; row 64 = sum_k exp = Z)
  normalize: DVE reciprocal(Z) -> GPSIMD partition_broadcast -> DVE mul -> c_T
  y = c_T.T @ WoT_g  (per-core partial, fp32 out)
"""

import numpy as np
import ml_dtypes

import concourse.bacc as bacc
import concourse.bass as bass
import concourse.mybir as mybir
import concourse.tile as tile
from concourse.bass_utils import run_bass_kernel_spmd

B, N, D = 4, 2048, 1024
H, HD, MB = 16, 64, 32
NCORES = 8
HPG = 8              # heads per group (per core)
GD = HPG * HD        # 512 head dims per group
SCALE = float(1.0 / np.sqrt(MB))

bf16 = mybir.dt.bfloat16
f32 = mybir.dt.float32
BF = ml_dtypes.bfloat16
AF = mybir.ActivationFunctionType

KT_X = D // 128      # 8 contraction tiles over d_model
NT = N // 512        # 4 column tiles of 512 over sequence
MT_QK = GD // 128    # 4 partition tiles of QT/KT
NT128 = N // 128     # 16 row tiles of 128 over sequence
KT_C = GD // 128     # 4 contraction tiles over group head dims

TRACE = False        # set by test.py for profiling runs
TRACE_KW = {}
LAST_RESULTS = None
PHASE_LIMIT = "full"  # "qkv" | "attn" | "full" — for sim phase ablation


def build(reps=1):
    nc = bacc.Bacc("TRN2", target_bir_lowering=False, debug=False,
                   num_devices=NCORES)
    xt = nc.dram_tensor("xt", [D, N], bf16, kind="ExternalInput").ap()
    wq = nc.dram_tensor("wq", [D, GD], bf16, kind="ExternalInput").ap()
    wk = nc.dram_tensor("wk", [D, GD], bf16, kind="ExternalInput").ap()
    wv = nc.dram_tensor("wv", [D, GD], bf16, kind="ExternalInput").ap()
    wenc = nc.dram_tensor("wenc", [128, HPG * MB], bf16, kind="ExternalInput").ap()
    wo = nc.dram_tensor("wo", [GD, D], bf16, kind="ExternalInput").ap()
    y = nc.dram_tensor("y", [N, D], f32, kind="ExternalOutput").ap()

    with tile.TileContext(nc) as tc:
        with (
            tc.tile_pool(name="xtp", bufs=KT_X) as xt_pool,
            tc.tile_pool(name="wp", bufs=3 * KT_X) as w_pool,
            tc.tile_pool(name="wop", bufs=KT_C) as wo_pool,
            tc.tile_pool(name="wencp", bufs=1) as wenc_pool,
            tc.tile_pool(name="qkp", bufs=2 * MT_QK) as qk_pool,
            tc.tile_pool(name="encp", bufs=4) as enc_pool,
            tc.tile_pool(name="vp", bufs=NT128) as v_pool,
            tc.tile_pool(name="expp", bufs=6) as exp_pool,
            tc.tile_pool(name="ctp", bufs=KT_C) as ct_pool,
            tc.tile_pool(name="smallp", bufs=8) as small_pool,
            tc.tile_pool(name="yp", bufs=3) as y_pool,
            tc.tile_pool(name="stp", bufs=2, space="PSUM") as st_pool,
            tc.tile_pool(name="bankp", bufs=4, space="PSUM") as bank_pool,
        ):
          for _rep in range(reps):
            # ---- input loads -------------------------------------------------
            xt_sb = []
            for k in range(KT_X):
                t = xt_pool.tile([128, N], bf16, tag="xt")
                nc.sync.dma_start(t[:], xt[128 * k:128 * (k + 1), :])
                xt_sb.append(t)

            def load_w(w_ap):
                tiles = []
                for k in range(KT_X):
                    t = w_pool.tile([128, GD], bf16, tag="w")
                    nc.sync.dma_start(t[:], w_ap[128 * k:128 * (k + 1), :])
                    tiles.append(t)
                return tiles

            wq_sb = load_w(wq)
            wk_sb = load_w(wk)
            wv_sb = load_w(wv)
            wo_sb = []
            for k in range(KT_C):
                t = wo_pool.tile([128, D], bf16, tag="wo")
                nc.sync.dma_start(t[:], wo[128 * k:128 * (k + 1), :])
                wo_sb.append(t)
            wenc_sb = wenc_pool.tile([128, HPG * MB], bf16, tag="wenc")
            nc.sync.dma_start(wenc_sb[:], wenc[:, :])

            # ---- QT / KT: [512 dims, 2048 seq], dims on partitions ----------
            # Interleave Q/K m-tiles and emit each encoder quad as soon as its
            # two m-tiles exist, so the first exp can start early.
            qT_sb, kT_sb = [None] * MT_QK, [None] * MT_QK
            q_enc, k_enc = [None, None], [None, None]

            def qk_mtile_nt(wsb, t, mt, nt, name):
                ps = bank_pool.tile([128, 512], f32, tag="bank",
                                    name=f"ps_{name}_{nt}")
                for k in range(KT_X):
                    nc.tensor.matmul(
                        ps[:],
                        wsb[k][:, 128 * mt:128 * (mt + 1)],
                        xt_sb[k][:, 512 * nt:512 * (nt + 1)],
                        start=(k == 0), stop=(k == KT_X - 1),
                        skip_group_check=True,
                    )
                nc.vector.tensor_copy(t[:, 512 * nt:512 * (nt + 1)], ps[:])

            def qk_mtile(wsb, mt, name):
                t = qk_pool.tile([128, N], bf16, tag="qk", name=name)
                for nt in range(NT):
                    qk_mtile_nt(wsb, t, mt, nt, name)
                return t

            def encoder_nt(src, et, qd, nt, name):
                eps = bank_pool.tile([128, 512], f32, tag="bank",
                                     name=f"eps_{name}_{nt}")
                for r in range(4):
                    h = 4 * qd + r
                    e = 64 * (h % 2)
                    nc.tensor.matmul(
                        eps[32 * r:32 * (r + 1), :],
                        wenc_sb[e:e + 64, MB * h:MB * (h + 1)],
                        src[h // 2][e:e + 64, 512 * nt:512 * (nt + 1)],
                        start=True, stop=True,
                        tile_position=(e, 32 * r),
                        skip_group_check=True,
                    )
                nc.scalar.activation(et[:, 512 * nt:512 * (nt + 1)],
                                     eps[:], AF.Tanh)

            def encoder(src, qd, name):
                # [128, 2048]: 4 heads x 32 bits on partitions, via row+col
                # tiled K=64 matmuls, tanh per 512-slice from a 1-bank psum
                et = enc_pool.tile([128, N], bf16, tag="enc", name=name)
                for nt in range(NT):
                    encoder_nt(src, et, qd, nt, name)
                return et

            v_sb = [None] * NT128

            def ensure_v(nt):
                if v_sb[nt] is not None:
                    return v_sb[nt]
                t = v_pool.tile([128, HPG * 65], bf16, tag="v", name=f"v{nt}")
                ps = bank_pool.tile([128, 512], f32, tag="bank",
                                    name=f"ps_v{nt}")
                for k in range(KT_X):
                    nc.tensor.matmul(
                        ps[:],
                        xt_sb[k][:, 128 * nt:128 * (nt + 1)],
                        wv_sb[k][:],
                        start=(k == 0), stop=(k == KT_X - 1),
                        skip_group_check=True,
                    )
                vv = t[:, :].rearrange("p (h s) -> p h s", h=HPG)
                nc.vector.tensor_copy(
                    vv[:, :, 0:64],
                    ps[:, :].rearrange("p (h s) -> p h s", h=HPG),
                )
                nc.vector.memset(vv[:, :, 64:65], 1.0)
                v_sb[nt] = t
                return t

            # ---- c_T accumulator tiles: [512 head dims, 2048 seq] -----------
            ct_sb = [ct_pool.tile([128, N], bf16, tag="ct", name=f"ct{i}")
                     for i in range(KT_C)]

            def attention_pair(p, filler=None):
                """ST -> exp -> att.V for heads (2p, 2p+1), all q-tiles.

                st tiles are [128, 1024] (2 heads x 512 q), double-buffered,
                so ACT streams exps continuously; attV accumulates into 2
                psum banks per pair.  filler(qt) emits lower-priority PE work
                after each q-tile so it drains inside this pair's exp windows.
                """
                qd = p // 2
                for qt in range(NT):
                    att = [bank_pool.tile([65, 512], f32, tag="bank",
                                          name=f"att{p}_{qt}_{r}")
                           for r in range(2)]
                    for kt in range(NT128):
                        st = st_pool.tile([128, N // 2], f32, tag="st")
                        for r in range(2):
                            a = 2 * (p % 2) + r
                            nc.tensor.matmul(
                                st[:, 512 * r:512 * (r + 1)],
                                k_enc[qd][32 * a:32 * (a + 1), 128 * kt:128 * (kt + 1)],
                                q_enc[qd][32 * a:32 * (a + 1), 512 * qt:512 * (qt + 1)],
                                start=True, stop=True,
                                tile_position=(32 * a, 0),
                                skip_group_check=True,
                            )
                        ex = exp_pool.tile([128, N // 2], bf16, tag="exp")
                        nc.scalar.activation(ex[:], st[:], AF.Exp, scale=SCALE)
                        ensure_v(min(kt + 2, NT128 - 1))
                        for r in range(2):
                            h = 2 * p + r
                            nc.tensor.matmul(
                                att[r][:],
                                ensure_v(kt)[:, 65 * h:65 * h + 65],
                                ex[:, 512 * r:512 * (r + 1)],
                                start=(kt == 0), stop=(kt == NT128 - 1),
                                skip_group_check=True,
                            )
                    for r in range(2):
                        h = 2 * p + r
                        recip = small_pool.tile([1, 512], f32, tag="recip")
                        nc.vector.reciprocal(recip[:], att[r][64:65, :])
                        bc = small_pool.tile([64, 512], f32, tag="bc")
                        nc.gpsimd.partition_broadcast(bc[:], recip[:])
                        u = 64 * (h % 2)
                        if u == 0:
                            nc.vector.tensor_mul(
                                ct_sb[h // 2][0:64, 512 * qt:512 * (qt + 1)],
                                att[r][0:64, :], bc[:])
                        else:
                            tmp = small_pool.tile([64, 512], bf16, tag="tmp")
                            nc.vector.tensor_mul(tmp[:], att[r][0:64, :], bc[:])
                            nc.sync.dma_start(
                                ct_sb[h // 2][64:128, 512 * qt:512 * (qt + 1)],
                                tmp[:])
                    if filler is not None:
                        filler(qt)

            def out_proj_qt(qt):
                # y rows 512*qt .. 512*(qt+1): 4 m-tiles x 2 out-dim halves
                for mt in range(4 * qt, 4 * qt + 4):
                    for nt2 in range(2):
                        ps = bank_pool.tile([128, 512], f32, tag="bank",
                                            name=f"ps_y{mt}_{nt2}")
                        for k in range(KT_C):
                            nc.tensor.matmul(
                                ps[:],
                                ct_sb[k][:, 128 * mt:128 * (mt + 1)],
                                wo_sb[k][:, 512 * nt2:512 * (nt2 + 1)],
                                start=(k == 0), stop=(k == KT_C - 1),
                                skip_group_check=True,
                            )
                        yt = y_pool.tile([128, 512], f32, tag="y")
                        nc.vector.tensor_copy(yt[:], ps[:])
                        nc.sync.dma_start(
                            y[128 * mt:128 * (mt + 1),
                              512 * nt2:512 * (nt2 + 1)],
                            yt[:])

            # Emission order drives scheduler priority.  Ramp: emit QK m0/m1
            # and encoder quad 0 one 512-column slice at a time, k-enc first
            # (ST needs the full k_enc row but only q_enc's first slice), so
            # the first exp starts after ~3 slices.  The rest of QK/enc/V is
            # emitted inside pair 0/1's exp windows; pair 2/3's windows are
            # left for the output-projection chase.
            qT_sb[0] = qk_pool.tile([128, N], bf16, tag="qk", name="qT0")
            qT_sb[1] = qk_pool.tile([128, N], bf16, tag="qk", name="qT1")
            kT_sb[0] = qk_pool.tile([128, N], bf16, tag="qk", name="kT0")
            kT_sb[1] = qk_pool.tile([128, N], bf16, tag="qk", name="kT1")
            q_enc[0] = enc_pool.tile([128, N], bf16, tag="enc", name="qenc0")
            k_enc[0] = enc_pool.tile([128, N], bf16, tag="enc", name="kenc0")
            for nt in range(NT):
                qk_mtile_nt(wk_sb, kT_sb[0], 0, nt, "kT0")
                qk_mtile_nt(wk_sb, kT_sb[1], 1, nt, "kT1")
                encoder_nt(kT_sb, k_enc[0], 0, nt, "kenc0")
                qk_mtile_nt(wq_sb, qT_sb[0], 0, nt, "qT0")
                qk_mtile_nt(wq_sb, qT_sb[1], 1, nt, "qT1")
                encoder_nt(qT_sb, q_enc[0], 0, nt, "qenc0")
            for nt in range(2):
                ensure_v(nt)
            if PHASE_LIMIT == "qkv":
                continue

            qT_sb[2] = qk_pool.tile([128, N], bf16, tag="qk", name="qT2")
            qT_sb[3] = qk_pool.tile([128, N], bf16, tag="qk", name="qT3")
            kT_sb[2] = qk_pool.tile([128, N], bf16, tag="qk", name="kT2")
            kT_sb[3] = qk_pool.tile([128, N], bf16, tag="qk", name="kT3")
            q_enc[1] = enc_pool.tile([128, N], bf16, tag="enc", name="qenc1")
            k_enc[1] = enc_pool.tile([128, N], bf16, tag="enc", name="kenc1")

            def qk23_filler(nt):
                qk_mtile_nt(wk_sb, kT_sb[2], 2, nt, "kT2")
                qk_mtile_nt(wk_sb, kT_sb[3], 3, nt, "kT3")
                encoder_nt(kT_sb, k_enc[1], 1, nt, "kenc1")
                qk_mtile_nt(wq_sb, qT_sb[2], 2, nt, "qT2")
                qk_mtile_nt(wq_sb, qT_sb[3], 3, nt, "qT3")
                encoder_nt(qT_sb, q_enc[1], 1, nt, "qenc1")

            attention_pair(0)
            attention_pair(1, filler=qk23_filler)
            attention_pair(2)
            attention_pair(3, filler=out_proj_qt)
    nc.finalize()
    return nc


_nc_cache = None


def make_in_maps(x, Wq, Wk, Wv, We, Wo):
    xts = [np.ascontiguousarray(x[b].T).astype(BF) for b in range(B)]
    in_maps = []
    for c in range(NCORES):
        b, g = divmod(c, 2)
        gs = g * GD
        we_g = We[g * HPG:(g + 1) * HPG]          # [8, 64, 32]
        we_blk = np.ascontiguousarray(
            we_g.transpose(1, 0, 2).reshape(HD, HPG * MB))  # [64, 256]
        in_maps.append({
            "xt": xts[b],
            "wq": np.ascontiguousarray(Wq[gs:gs + GD, :].T).astype(BF),
            "wk": np.ascontiguousarray(Wk[gs:gs + GD, :].T).astype(BF),
            "wv": np.ascontiguousarray(Wv[gs:gs + GD, :].T).astype(BF),
            "wenc": np.concatenate([we_blk, we_blk], axis=0).astype(BF),
            "wo": np.ascontiguousarray(Wo[:, gs:gs + GD].T).astype(BF),
        })
    return in_maps


def kernel(**inputs):
    global _nc_cache, LAST_RESULTS
    x = np.asarray(inputs["x"], dtype=np.float32)
    Wq = np.asarray(inputs["Wq"], dtype=np.float32)
    Wk = np.asarray(inputs["Wk"], dtype=np.float32)
    Wv = np.asarray(inputs["Wv"], dtype=np.float32)
    We = np.asarray(inputs["W_enc"], dtype=np.float32)
    Wo = np.asarray(inputs["Wo"], dtype=np.float32)

    if _nc_cache is None:
        _nc_cache = build()
    nc = _nc_cache

    in_maps = make_in_maps(x, Wq, Wk, Wv, We, Wo)

    res = run_bass_kernel_spmd(
        nc, in_maps, core_ids=list(range(NCORES)),
        trace=TRACE, **TRACE_KW)
    LAST_RESULTS = res

    out = np.empty((B, N, D), dtype=np.float32)
    for b in range(B):
        out[b] = res.results[2 * b]["y"] + res.results[2 * b + 1]["y"]
    return out



# revision 20
# speedup vs baseline: 1.0425x; 1.0425x over previous
"""BREWA (bit-witness) attention on 8 TRN2 NeuronCores — v2.

Sharding: core c = (batch b, head-group g), b = c // 2, g = c % 2.  Each core
computes its batch's attention for 8 of the 16 heads plus the partial output
projection over those heads' Wo columns; the host sums the two partials.

v2 changes vs v1:
  - Fused witness encoders: q_enc = tanh(x @ (Wq_h^T @ W_enc_h)) computed
    directly (host-fused weights Wqe/Wke [1024, 256]), skipping the Q/K
    intermediates entirely: half the projection FLOPs and no QK PSUM
    evacuation.
  - exp offload: a fraction of score tiles use a Schraudolph fast-exp on the
    vector engine (bits = int16(A*s + B) bitcast to bf16), the rest run
    exact Exp on the scalar engine - the two engines pipeline score tiles
    in parallel.
  - Software-pipelined emission: ST(kt)/exp(kt) lead attV(kt-1) by one step,
    with enc/V/y "chase" work pumped into the exp windows.

Per-core dataflow (bf16 matmuls, fp32 PSUM):
  xt [1024, 2048]  --PE (8k x 2quad x 4nt)-->  enc psum --ACT tanh-->
    q_enc/k_enc [128, 2048] per quad (4 heads x 32 bits on partitions)
  V: xt^T @ Wv -> v_sb [128, 8*65] per kt (64 dims + ones col per head)
  per (pair, qt512, kt128):
    ST: 2 row-tiled (K=32) MMs -> st psum [128, 1024]
    exp: ACT Exp (exact) or DVE Schraudolph (fast) -> ex bf16
    attV: 2 MMs (M=65) accumulate att psum [65, 512]; row 64 = Z
  normalize: recip(Z) -> Pool broadcast -> DVE mul -> ct bf16 [128, 2048]
  y = ct^T @ Wo per qt (chased), fp32 out.
"""

import numpy as np
import ml_dtypes

import concourse.bacc as bacc
import concourse.bass as bass
import concourse.mybir as mybir
import concourse.tile as tile
from concourse.bass_utils import run_bass_kernel_spmd

B, N, D = 4, 2048, 1024
H, HD, MB = 16, 64, 32
NCORES = 8
HPG = 8              # heads per group (per core)
GD = HPG * HD        # 512 head dims per group
SCALE = float(1.0 / np.sqrt(MB))

# Schraudolph fast-exp constants (bf16 bit trick): bits = int16(A*s + B)
FE_A = float(128.0 * np.log2(np.e) * SCALE)
FE_B = float(128.0 * (127.0 - 0.045))

bf16 = mybir.dt.bfloat16
f32 = mybir.dt.float32
i16 = mybir.dt.int16
BF = ml_dtypes.bfloat16
AF = mybir.ActivationFunctionType

KT_X = D // 128      # 8 contraction tiles over d_model
NT = N // 512        # 4 column tiles of 512 over sequence
NT128 = N // 128     # 16 row tiles of 128 over sequence
KT_C = GD // 128     # 4 contraction tiles over group head dims

# kt steps whose exp runs on the vector engine (Schraudolph fast-exp)
DVE_KTS = frozenset((2, 5, 8, 11, 14))

TRACE = False        # set by test.py for profiling runs
TRACE_KW = {}
LAST_RESULTS = None


def build(reps=1):
    nc = bacc.Bacc("TRN2", target_bir_lowering=False, debug=False,
                   num_devices=NCORES)
    xt = nc.dram_tensor("xt", [D, N], bf16, kind="ExternalInput").ap()
    # we = [wke | wqe], each 2 quads x (4 heads x 32 bits)
    we = nc.dram_tensor("we", [D, 512], bf16, kind="ExternalInput").ap()
    wv = nc.dram_tensor("wv", [D, GD], bf16, kind="ExternalInput").ap()
    wo = nc.dram_tensor("wo", [GD, D], bf16, kind="ExternalInput").ap()
    y = nc.dram_tensor("y", [N, D], f32, kind="ExternalOutput").ap()

    with tile.TileContext(nc) as tc:
        with (
            tc.tile_pool(name="xtp", bufs=KT_X) as xt_pool,
            tc.tile_pool(name="wep", bufs=KT_X) as we_pool,
            tc.tile_pool(name="wvp", bufs=KT_X) as wv_pool,
            tc.tile_pool(name="wop", bufs=KT_C) as wo_pool,
            tc.tile_pool(name="encp", bufs=4) as enc_pool,
            tc.tile_pool(name="vp", bufs=NT128) as v_pool,
            tc.tile_pool(name="expp", bufs=8) as ex_pool,
            tc.tile_pool(name="ctp", bufs=KT_C) as ct_pool,
            tc.tile_pool(name="smallp", bufs=8) as small_pool,
            tc.tile_pool(name="yp", bufs=3) as y_pool,
            tc.tile_pool(name="stp", bufs=2, space="PSUM") as st_pool,
            tc.tile_pool(name="attp", bufs=3, space="PSUM") as att_pool,
            tc.tile_pool(name="chasep", bufs=1, space="PSUM") as chase_pool,
        ):
          for _rep in range(reps):
            # ---- input loads, split across the SP and ACT HWDGE rings ------
            # (one ring serializes at ~200 GB/s; the ramp needs ~6.5 MB)
            # Few, big DMAs (ring-issue cost is per-DMA), split over the SP
            # and ACT HWDGE rings; priority order we -> xt -> wv -> wo.
            def ring(k):
                return nc.sync if k % 2 == 0 else nc.scalar

            we_sb, xt_sb, wv_sb, wo_sb = [], [], [], []
            for k in range(KT_X):
                t = we_pool.tile([128, 512], bf16, tag="we", name=f"we{k}")
                ring(k).dma_start(t[:], we[128 * k:128 * (k + 1), :])
                we_sb.append(t)
            for k in range(KT_X):
                t = xt_pool.tile([128, N], bf16, tag="xt", name=f"xt{k}")
                ring(k).dma_start(t[:], xt[128 * k:128 * (k + 1), :])
                xt_sb.append(t)
            for k in range(KT_X):
                t = wv_pool.tile([128, GD], bf16, tag="wv", name=f"wv{k}")
                ring(k).dma_start(t[:], wv[128 * k:128 * (k + 1), :])
                wv_sb.append(t)
            for k in range(KT_C):
                t = wo_pool.tile([128, D], bf16, tag="wo", name=f"wo{k}")
                ring(k).dma_start(t[:], wo[128 * k:128 * (k + 1), :])
                wo_sb.append(t)

            # ---- fused encoders: enc[quad] = tanh(xt^T @ We) --------------
            q_enc = [enc_pool.tile([128, N], bf16, tag="enc", name=f"qenc{i}")
                     for i in range(2)]
            k_enc = [enc_pool.tile([128, N], bf16, tag="enc", name=f"kenc{i}")
                     for i in range(2)]

            def enc_group(dst, wtag, qd, nt):
                ps = chase_pool.tile([128, 512], f32, tag="chase",
                                     name=f"ps_{wtag}{qd}_{nt}")
                off = (0 if wtag == "wke" else 256) + 128 * qd
                for k in range(KT_X):
                    nc.tensor.matmul(
                        ps[:],
                        we_sb[k][:, off:off + 128],
                        xt_sb[k][:, 512 * nt:512 * (nt + 1)],
                        start=(k == 0), stop=(k == KT_X - 1),
                        skip_group_check=True,
                    )
                nc.scalar.activation(dst[qd][:, 512 * nt:512 * (nt + 1)],
                                     ps[:], AF.Tanh)

            # ---- V tiles: [128 seq, 8 heads x (64 dims + ones)] -----------
            v_sb = [None] * NT128

            def ensure_v(kt):
                if v_sb[kt] is not None:
                    return v_sb[kt]
                t = v_pool.tile([128, HPG * 65], bf16, tag="v", name=f"v{kt}")
                ps = chase_pool.tile([128, 512], f32, tag="chase",
                                     name=f"ps_v{kt}")
                for k in range(KT_X):
                    nc.tensor.matmul(
                        ps[:],
                        xt_sb[k][:, 128 * kt:128 * (kt + 1)],
                        wv_sb[k][:],
                        start=(k == 0), stop=(k == KT_X - 1),
                        skip_group_check=True,
                    )
                vv = t[:, :].rearrange("p (h s) -> p h s", h=HPG)
                nc.vector.tensor_copy(
                    vv[:, :, 0:64],
                    ps[:, :].rearrange("p (h s) -> p h s", h=HPG),
                )
                nc.vector.memset(vv[:, :, 64:65], 1.0)
                v_sb[kt] = t
                return t

            # ---- c_T accumulator tiles: ct[p] rows = heads (2p, 2p+1) -----
            ct_sb = [ct_pool.tile([128, N], bf16, tag="ct", name=f"ct{i}")
                     for i in range(KT_C)]

            def y_group(mt, half, pool=None):
                ps = (pool or chase_pool).tile([128, 512], f32,
                                               tag="chase" if pool is None
                                               else "att",
                                               name=f"ps_y{mt}_{half}")
                for k in range(KT_C):
                    nc.tensor.matmul(
                        ps[:],
                        ct_sb[k][:, 128 * mt:128 * (mt + 1)],
                        wo_sb[k][:, 512 * half:512 * (half + 1)],
                        start=(k == 0), stop=(k == KT_C - 1),
                        skip_group_check=True,
                    )
                yt = y_pool.tile([128, 512], f32, tag="y")
                nc.vector.tensor_copy(yt[:], ps[:])
                nc.sync.dma_start(
                    y[128 * mt:128 * (mt + 1), 512 * half:512 * (half + 1)],
                    yt[:])

            # ---- chase list: filler emitted inside attention windows ------
            chase = []
            chase_pos = [0]

            def pump(n=1):
                end = min(chase_pos[0] + n, len(chase))
                while chase_pos[0] < end:
                    chase[chase_pos[0]]()
                    chase_pos[0] += 1

            # qt-major loop: kenc0 tail first (qt0-p0 consumes kt 4..15),
            # then quad-1 encoders (needed from qt0-p2), then later q slices.
            for nt in range(1, NT):
                chase.append(lambda nt=nt: enc_group(k_enc, "wke", 0, nt))
            for nt in range(NT):
                chase.append(lambda nt=nt: enc_group(k_enc, "wke", 1, nt))
            chase.append(lambda: enc_group(q_enc, "wqe", 1, 0))
            for nt in range(1, NT):
                chase.append(lambda nt=nt: enc_group(q_enc, "wqe", 0, nt))
                chase.append(lambda nt=nt: enc_group(q_enc, "wqe", 1, nt))

            # ---- ramp -----------------------------------------------------
            enc_group(k_enc, "wke", 0, 0)
            enc_group(q_enc, "wqe", 0, 0)
            ensure_v(0)
            ensure_v(1)

            # ---- attention: one flattened software-pipelined stream -------
            # steps (qt, p, kt); ST/exp lead their attV by 2 steps so the PE
            # queue never gates on the exp latency; att psum tiles rotate
            # through 3 banks so block i+1 overlaps block i's normalize.
            atts = {}

            def emit_st_exp(qt, p, kt):
                qd, pp = p // 2, p % 2
                st = st_pool.tile([128, N // 2], f32, tag="st")
                for r in range(2):
                    a = 2 * pp + r
                    nc.tensor.matmul(
                        st[:, 512 * r:512 * (r + 1)],
                        k_enc[qd][32 * a:32 * (a + 1),
                                  128 * kt:128 * (kt + 1)],
                        q_enc[qd][32 * a:32 * (a + 1),
                                  512 * qt:512 * (qt + 1)],
                        start=True, stop=True,
                        tile_position=(32 * a, 0),
                        skip_group_check=True,
                    )
                ex = ex_pool.tile([128, N // 2], bf16, tag="ex")
                if kt in DVE_KTS:
                    nc.vector.tensor_scalar(
                        ex[:, :].bitcast(i16), st[:], FE_A, FE_B,
                        mybir.AluOpType.mult, mybir.AluOpType.add)
                else:
                    nc.scalar.activation(ex[:], st[:], AF.Exp, scale=SCALE)
                return ex

            def emit_attv(qt, p, kt, ex):
                ensure_v(min(kt + 2, NT128 - 1))
                att = atts[(qt, p)]
                for r in range(2):
                    h = 2 * p + r
                    nc.tensor.matmul(
                        att[r][0:65, :],
                        ensure_v(kt)[:, 65 * h:65 * h + 65],
                        ex[:, 512 * r:512 * (r + 1)],
                        start=(kt == 0), stop=(kt == NT128 - 1),
                        skip_group_check=True,
                    )
                if kt == NT128 - 1:
                    normalize(qt, p)

            def normalize(qt, p):
                # -> ct (odd head needs a partition shift, done via DMA)
                att = atts.pop((qt, p))
                for r in range(2):
                    recip = small_pool.tile([1, 512], f32, tag="recip")
                    nc.vector.reciprocal(recip[:], att[r][64:65, :])
                    bc = small_pool.tile([64, 512], f32, tag="bc")
                    nc.gpsimd.partition_broadcast(bc[:], recip[:])
                    if r == 0:
                        nc.vector.tensor_mul(
                            ct_sb[p][0:64, 512 * qt:512 * (qt + 1)],
                            att[r][0:64, :], bc[:])
                    else:
                        tmp = small_pool.tile([64, 512], bf16, tag="tmp")
                        nc.vector.tensor_mul(tmp[:], att[r][0:64, :], bc[:])
                        nc.sync.dma_start(
                            ct_sb[p][64:128, 512 * qt:512 * (qt + 1)],
                            tmp[:])
                if p == 3:
                    for j, mt in enumerate(range(4 * qt, 4 * qt + 4)):
                        for half in range(2):
                            # final qt: spread y psums over two pools so the
                            # tail is not serialized on one chase bank
                            pl = att_pool if qt == NT - 1 and (j + half) % 2 \
                                else None
                            chase.append(lambda mt=mt, half=half, pl=pl:
                                         y_group(mt, half, pl))

            steps = [(qt, p, kt)
                     for qt in range(NT) for p in range(4)
                     for kt in range(NT128)]
            LAG = 2
            pipeline = []
            for i, (qt, p, kt) in enumerate(steps):
                if kt == 0:
                    atts[(qt, p)] = [
                        att_pool.tile([128, 512], f32, tag="att",
                                      name=f"att{p}_{qt}_{r}")
                        for r in range(2)]
                ex = emit_st_exp(qt, p, kt)
                pipeline.append((qt, p, kt, ex))
                if len(pipeline) > LAG:
                    emit_attv(*pipeline.pop(0))
                if i % 3 == 0:
                    pump(1)
            while pipeline:
                emit_attv(*pipeline.pop(0))
            pump(len(chase))
    nc.finalize()
    return nc


_nc_cache = None


def make_in_maps(x, Wq, Wk, Wv, We, Wo):
    xts = [np.ascontiguousarray(x[b].T).astype(BF) for b in range(B)]

    def fuse(W, g):
        # [1024, 256]: cols = quad*128 + head_in_quad*32 + bit
        blocks = []
        for j in range(HPG):
            hh = HPG * g + j
            blocks.append(W[HD * hh:HD * (hh + 1), :].T @ We[hh])
        return np.concatenate(blocks, axis=1).astype(BF)

    in_maps = []
    for c in range(NCORES):
        b, g = divmod(c, 2)
        gs = g * GD
        in_maps.append({
            "xt": xts[b],
            "we": np.concatenate([fuse(Wk, g), fuse(Wq, g)],
                                 axis=1).astype(BF),
            "wv": np.ascontiguousarray(Wv[gs:gs + GD, :].T).astype(BF),
            "wo": np.ascontiguousarray(Wo[:, gs:gs + GD].T).astype(BF),
        })
    return in_maps


def kernel(**inputs):
    global _nc_cache, LAST_RESULTS
    x = np.asarray(inputs["x"], dtype=np.float32)
    Wq = np.asarray(inputs["Wq"], dtype=np.float32)
    Wk = np.asarray(inputs["Wk"], dtype=np.float32)
    Wv = np.asarray(inputs["Wv"], dtype=np.float32)
    We = np.asarray(inputs["W_enc"], dtype=np.float32)
    Wo = np.asarray(inputs["Wo"], dtype=np.float32)

    if _nc_cache is None:
        _nc_cache = build()
    nc = _nc_cache

    in_maps = make_in_maps(x, Wq, Wk, Wv, We, Wo)

    res = run_bass_kernel_spmd(
        nc, in_maps, core_ids=list(range(NCORES)),
        trace=TRACE, **TRACE_KW)
    LAST_RESULTS = res

    out = np.empty((B, N, D), dtype=np.float32)
    for b in range(B):
        out[b] = res.results[2 * b]["y"] + res.results[2 * b + 1]["y"]
    return out
